# revision 16
# baseline (speedup 1.0000x reference)
"""AttentionAvg kernel for 8 Trainium2 NeuronCores — v2.

Reference (per batch b):
    q = x @ Wq^T + bq; k = x @ Wk^T + bk          (t, d)
    s = q @ k^T / sqrt(d);  s[:, j] = -1e9 where mask[j] == 0
    w = softmax(s, axis=-1);  out[b] = sum_t x[t] * w[t, t]

Only the softmax *diagonal* is needed.  Expanding the scores,
    s[q, k] = x_q^T A x_k + u[q] + v[k] + c,
    A = Wq^T Wk / sqrt(d),   v = Wk^T bq / sqrt(d),
and the row-constant terms u[q] + c cancel in
    w[t, t] = exp(s_tt) / sum_k exp(s_tk).
So ONE projection  Y = X A  with v folded in as the per-partition
activation bias (Y_q = A^T x_q + v  =>  Y_q . x_k = x_q^T A x_k + v[k])
replaces the two d x d Q/K projections of the naive pipeline.

Masked rows/keys are dropped by a HOST-side gather of the unmasked rows,
zero-padded to a multiple of 128 (Tg ~ T/2 for this mask, shrinking the
dominant (t, t, d) matmul ~4x).  Zero-padded COLUMNS contribute exactly
exp(0) = 1 to every row sum — corrected by subtracting n_pad from Z on
device.  Zero-padded ROWS get harmless finite weights that multiply
all-zero x rows in the final matvec.  The host also pre-transposes and
casts everything, so the device performs no gathers and no transposes;
XT arrives chunked by plain DMA and the PE can start almost immediately.

Per-core device pipeline (Tile framework):
  1. per k-chunk: YT[:, :, chunk] = A^T @ XT(chunk) + v     (PE + ACT)
  2. wavefront S(ib, jc) = YT(ib)^T @ XT(jc) in PSUM; ACT exp -> SBUF
     with accum_out row-sums into Zbig; the diagonal block is extracted
     with a fused tensor_tensor_reduce against an identity mask.
  3. w = diag * 1/(Z - n_pad); out += w^T @ X via accumulating PE matvec
     against host-gathered x rows (bf16), PSUM -> SBUF -> DRAM.

Sharding: data-parallel over batch, one batch row per core (8 == 8).
"""

import math
import sys

import numpy as np

for _p in ("/opt/trn_rl_repo",):
    if _p not in sys.path:
        sys.path.insert(0, _p)

import ml_dtypes  # noqa: E402

import concourse.bass as bass  # noqa: E402
from concourse import bacc  # noqa: E402
import concourse.mybir as mybir  # noqa: E402
import concourse.tile as tile  # noqa: E402

B, T, D = 8, 4096, 768
P = 128
DT = D // P  # 6 contraction tiles
CH = 512  # free-dim chunk width (one PSUM bank of fp32)
HD = D // 2  # finalize matvec split (<=512 per PSUM bank)
NCORES = 8
SCALE = 1.0 / math.sqrt(D)

F32 = mybir.dt.float32
BF16 = mybir.dt.bfloat16
BF = ml_dtypes.bfloat16


def _chunks(n, width):
    """Remainder-FIRST chunking: smallest chunk leads, shrinking the
    DMA->first-matmul head latency."""
    out = []
    c0 = 0
    rem = n % width
    if rem:
        out.append((0, rem))
        c0 = rem
    while c0 < n:
        out.append((c0, width))
        c0 += width
    return out


def build_graph(nc, Tg):
    """Emit the per-core graph for gathered/padded length Tg (multiple of P)."""
    JB = Tg // P
    chunks = _chunks(Tg, CH)

    IC = len(chunks)

    xt = nc.declare_dram_parameter("xt", [DT, P, Tg], BF16, isOutput=False)
    aw = nc.declare_dram_parameter("aw", [DT, P, D], BF16, isOutput=False)
    vb = nc.declare_dram_parameter("vb", [P, DT], F32, isOutput=False)
    idf = nc.declare_dram_parameter("idf", [P, P], F32, isOutput=False)
    npz = nc.declare_dram_parameter("npz", [P, JB], F32, isOutput=False)
    xg = nc.declare_dram_parameter("xg", [JB, P, D], BF16, isOutput=False)
    out = nc.declare_dram_parameter("out", [1, D], F32, isOutput=True)

    with tile.TileContext(nc) as tc:
        with (
            tc.tile_pool(name="singles", bufs=1) as singles,
            tc.tile_pool(name="spool", bufs=4) as spool,
            tc.tile_pool(name="stats", bufs=6) as stats,
            tc.tile_pool(name="psS", bufs=6, space="PSUM") as psS,
            tc.tile_pool(name="psO", bufs=1, space="PSUM") as psO,
        ):
            # ---- resident tensors ----
            XT = singles.tile([P, DT, Tg], BF16, tag="XT")
            YT = singles.tile([P, DT, Tg], BF16, tag="YT")
            XG = singles.tile([P, JB, D], BF16, tag="XG")
            AW = singles.tile([P, DT, D], BF16, tag="AW")
            VB = singles.tile([P, DT], F32, tag="VB")
            identity = singles.tile([P, P], F32, tag="ident")
            # [jc, ib] layout; the extra IC row holds -n_pad so the plain
            # row-sum over jc comes out already pad-corrected
            Zbig = singles.tile([P, IC + 1, JB], F32, tag="Zbig")
            diag_cols = singles.tile([P, JB], F32, tag="diag_cols")

            # ---- DMA issue: A first (needed by every Y matmul), then XT
            # chunks round-robin over queues, small singles, then xg rows
            # (only needed at finalize). ----
            qs = (nc.sync, nc.scalar)
            for ei in range(DT):
                qs[ei % 2].dma_start(AW[:, ei, :], aw[ei, :, :])
            qi = 0
            for c0, w in chunks:
                for ei in range(DT):
                    qs[qi % len(qs)].dma_start(
                        XT[:, ei, c0 : c0 + w], xt[ei, :, c0 : c0 + w]
                    )
                    qi += 1
            nc.scalar.dma_start(VB, vb[:, :])
            nc.scalar.dma_start(Zbig[:, IC, :], npz[:, :])
            nc.scalar.dma_start(identity, idf[:, :])
            # xg rows are needed only at finalize: keep them off the
            # critical queues and gate them behind early YT progress so
            # they don't steal HBM bandwidth from XT/AW at the head
            xg_gate = singles.tile([P, 1], BF16, tag="xg_gate")

            def emit_xg_dmas():
                nc.gpsimd.tensor_copy(out=xg_gate, in_=YT[:, 0, 0:1])
                for ib in range(JB):
                    nc.gpsimd.dma_start(XG[:, ib, :], xg[ib, :, :])

            po1 = psO.tile([1, HD], F32, tag="po1")
            po2 = psO.tile([1, HD], F32, tag="po2")

            def emit_y(c0, w):
                for eo in range(DT):
                    ps = psS.tile([P, CH], F32, tag="psS")
                    for ei in range(DT):
                        nc.tensor.matmul(
                            ps[:, :w],
                            lhsT=AW[:, ei, eo * P : (eo + 1) * P],
                            rhs=XT[:, ei, c0 : c0 + w],
                            start=(ei == 0),
                            stop=(ei == DT - 1),
                        )
                    nc.scalar.activation(
                        out=YT[:, eo, c0 : c0 + w],
                        in_=ps[:, :w],
                        func=mybir.ActivationFunctionType.Identity,
                        bias=VB[:, eo : eo + 1],
                        scale=1.0,
                    )

            def emit_s(ib, jc):
                c0, w = chunks[jc]
                ps = psS.tile([P, CH], F32, tag="psS")
                for et in range(DT):
                    nc.tensor.matmul(
                        ps[:, :w],
                        lhsT=YT[:, et, ib * P : (ib + 1) * P],
                        rhs=XT[:, et, c0 : c0 + w],
                        start=(et == 0),
                        stop=(et == DT - 1),
                    )
                e_sb = spool.tile([P, CH], F32, tag="esb")
                nc.scalar.activation(
                    out=e_sb[:, :w],
                    in_=ps[:, :w],
                    func=mybir.ActivationFunctionType.Exp,
                )
                nc.vector.reduce_sum(
                    Zbig[:, jc, ib : ib + 1], e_sb[:, :w], axis=mybir.AxisListType.X
                )
                dj = ib * P
                if c0 <= dj < c0 + w:
                    off = dj - c0
                    dsc = spool.tile([P, P], F32, tag="dsc")
                    nc.vector.tensor_mul(dsc, e_sb[:, off : off + P], identity)
                    nc.vector.reduce_sum(
                        diag_cols[:, ib : ib + 1], dsc, axis=mybir.AxisListType.X
                    )

            fin_n = [0]

            def emit_finalize(ib):
                z = stats.tile([P, 1], F32, tag="z")
                nc.vector.reduce_sum(
                    z, Zbig[:, :, ib : ib + 1], axis=mybir.AxisListType.XY
                )
                rz = stats.tile([P, 1], F32, tag="rz")
                nc.vector.reciprocal(rz, z)
                wcol = stats.tile([P, 1], BF16, tag="wcol")
                nc.vector.tensor_mul(wcol, diag_cols[:, ib : ib + 1], rz)
                for po, sl in ((po1, slice(0, HD)), (po2, slice(HD, D))):
                    nc.tensor.matmul(
                        po,
                        lhsT=wcol,
                        rhs=XG[:, ib, sl],
                        start=(fin_n[0] == 0),
                        stop=(fin_n[0] == JB - 1),
                    )
                fin_n[0] += 1

            # ---- wavefront: per chunk s compute YT(s), then all S(ib, jc)
            # with max(block(ib), jc) == s ----
            last = len(chunks) - 1
            for s, (c0, w) in enumerate(chunks):
                emit_y(c0, w)
                if s == 1:
                    emit_xg_dmas()
                sb0 = c0 // P
                sb1 = (c0 + w + P - 1) // P
                for ib in range(sb0, sb1):
                    for jc in range(s + 1):
                        emit_s(ib, jc)
                    if s == last:
                        emit_finalize(ib)
                for ib in range(0, sb0):
                    emit_s(ib, s)
                    if s == last:
                        emit_finalize(ib)

            out_sb = singles.tile([1, D], F32, tag="out_sb")
            nc.vector.tensor_copy(out=out_sb[:, :HD], in_=po1)
            nc.vector.tensor_copy(out=out_sb[:, HD:], in_=po2)
            nc.sync.dma_start(out[:, :], out_sb)

    return nc


def prepare_host_inputs(inputs, mask):
    """Per-batch gather + zero-pad to the common padded length Tg."""
    idxs, counts = [], []
    for b in range(B):
        nz = np.nonzero(mask[b])[0]
        idxs.append(nz)
        counts.append(len(nz))
    Tg = max(max(counts), 1)
    Tg = ((Tg + P - 1) // P) * P
    return Tg, idxs, counts


def kernel(inputs, mask, Wq_w, Wq_b, Wk_w, Wk_b, qk_bf16=True, _trace=False):
    from concourse.bass_utils import run_bass_kernel_spmd

    inputs = np.ascontiguousarray(inputs, np.float32)
    mask = np.asarray(mask)
    Tg, idxs, counts = prepare_host_inputs(inputs, mask)
    JB = Tg // P

    nc = bacc.Bacc()
    build_graph(nc, Tg)
    nc.compile()

    # s * Wq^T Wk  and  s * Wk^T bq  (row-constant score terms cancel)
    A = (np.asarray(Wq_w, np.float32).T @ np.asarray(Wk_w, np.float32)) * SCALE
    vvec = (np.asarray(Wk_w, np.float32).T @ np.asarray(Wq_b, np.float32)) * SCALE
    aw_arr = np.ascontiguousarray(A.astype(BF).reshape(DT, P, D))
    vb_arr = np.ascontiguousarray(vvec.reshape(DT, P).T)
    idf = np.eye(P, dtype=np.float32)

    in_maps = []
    for b in range(B):
        n = counts[b]
        xg_f = np.zeros((Tg, D), np.float32)
        if n:
            xg_f[:n] = inputs[b][idxs[b]]
        xg_bf = xg_f.astype(BF)
        xt_arr = np.ascontiguousarray(xg_bf.T.reshape(DT, P, Tg))
        xg_arr = xg_bf.reshape(JB, P, D)
        in_maps.append(
            {
                "xt": xt_arr,
                "aw": aw_arr,
                "vb": vb_arr,
                "idf": idf,
                "npz": np.full((P, JB), -float(Tg - n), np.float32),
                "xg": xg_arr,
            }
        )

    res = run_bass_kernel_spmd(
        nc, in_maps, core_ids=list(range(NCORES)), trace=_trace
    )
    out = np.stack([res.results[b]["out"][0] for b in range(B)], axis=0)

    # degenerate all-masked batch: softmax over a constant row is uniform
    for b in range(B):
        if counts[b] == 0:
            out[b] = inputs[b].mean(axis=0)

    if _trace:
        return out, res
    return out


# revision 21
# speedup vs baseline: 1.0292x; 1.0292x over previous
"""AttentionAvg kernel for 8 Trainium2 NeuronCores — v2.

Reference (per batch b):
    q = x @ Wq^T + bq; k = x @ Wk^T + bk          (t, d)
    s = q @ k^T / sqrt(d);  s[:, j] = -1e9 where mask[j] == 0
    w = softmax(s, axis=-1);  out[b] = sum_t x[t] * w[t, t]

Only the softmax *diagonal* is needed.  Expanding the scores,
    s[q, k] = x_q^T A x_k + u[q] + v[k] + c,
    A = Wq^T Wk / sqrt(d),   v = Wk^T bq / sqrt(d),
and the row-constant terms u[q] + c cancel in
    w[t, t] = exp(s_tt) / sum_k exp(s_tk).
So ONE projection  Y = X A  with v folded in as the per-partition
activation bias (Y_q = A^T x_q + v  =>  Y_q . x_k = x_q^T A x_k + v[k])
replaces the two d x d Q/K projections of the naive pipeline.

Masked rows/keys are dropped by a HOST-side gather of the unmasked rows,
zero-padded to a multiple of 128 (Tg ~ T/2 for this mask, shrinking the
dominant (t, t, d) matmul ~4x).  Zero-padded COLUMNS contribute exactly
exp(0) = 1 to every row sum — corrected by subtracting n_pad from Z on
device.  Zero-padded ROWS get harmless finite weights that multiply
all-zero x rows in the final matvec.  The host also pre-transposes and
casts everything, so the device performs no gathers and no transposes;
XT arrives chunked by plain DMA and the PE can start almost immediately.

Per-core device pipeline (Tile framework):
  1. per k-chunk: YT[:, :, chunk] = A^T @ XT(chunk) + v     (PE + ACT)
  2. wavefront S(ib, jc) = YT(ib)^T @ XT(jc) in PSUM; ACT exp -> SBUF
     with accum_out row-sums into Zbig; the diagonal block is extracted
     with a fused tensor_tensor_reduce against an identity mask.
  3. w = diag * 1/(Z - n_pad); out += w^T @ X via accumulating PE matvec
     against host-gathered x rows (bf16), PSUM -> SBUF -> DRAM.

Sharding: data-parallel over batch, one batch row per core (8 == 8).
"""

import math
import sys

import numpy as np

for _p in ("/opt/trn_rl_repo",):
    if _p not in sys.path:
        sys.path.insert(0, _p)

import ml_dtypes  # noqa: E402

import concourse.bass as bass  # noqa: E402
from concourse import bacc  # noqa: E402
import concourse.mybir as mybir  # noqa: E402
import concourse.tile as tile  # noqa: E402

B, T, D = 8, 4096, 768
P = 128
DT = D // P  # 6 contraction tiles
CH = 512  # free-dim chunk width (one PSUM bank of fp32)
HD = D // 2  # finalize matvec split (<=512 per PSUM bank)
NCORES = 8
SCALE = 1.0 / math.sqrt(D)

F32 = mybir.dt.float32
BF16 = mybir.dt.bfloat16
BF = ml_dtypes.bfloat16


def _chunks(n, width):
    """Remainder-FIRST chunking: smallest chunk leads, shrinking the
    DMA->first-matmul head latency."""
    out = []
    c0 = 0
    rem = n % width
    if rem:
        out.append((0, rem))
        c0 = rem
    while c0 < n:
        out.append((c0, width))
        c0 += width
    return out


def build_graph(nc, Tg):
    """Emit the per-core graph for gathered/padded length Tg (multiple of P)."""
    JB = Tg // P
    chunks = _chunks(Tg, CH)

    IC = len(chunks)

    xt = nc.declare_dram_parameter("xt", [DT, P, Tg], BF16, isOutput=False)
    aw = nc.declare_dram_parameter("aw", [P, DT * D], BF16, isOutput=False)
    vb = nc.declare_dram_parameter("vb", [P, DT], F32, isOutput=False)
    idf = nc.declare_dram_parameter("idf", [P, P], F32, isOutput=False)
    npz = nc.declare_dram_parameter("npz", [P, JB], F32, isOutput=False)
    xg = nc.declare_dram_parameter("xg", [JB, P, D], BF16, isOutput=False)
    out = nc.declare_dram_parameter("out", [1, D], F32, isOutput=True)

    with tile.TileContext(nc) as tc:
        with (
            tc.tile_pool(name="singles", bufs=1) as singles,
            tc.tile_pool(name="spool", bufs=4) as spool,
            tc.tile_pool(name="stats", bufs=6) as stats,
            tc.tile_pool(name="psS", bufs=6, space="PSUM") as psS,
            tc.tile_pool(name="psO", bufs=1, space="PSUM") as psO,
        ):
            # ---- resident tensors ----
            XT = singles.tile([P, DT, Tg], BF16, tag="XT")
            YT = singles.tile([P, DT, Tg], BF16, tag="YT")
            XG = singles.tile([P, JB, D], BF16, tag="XG")
            AW = singles.tile([P, DT, D], BF16, tag="AW")
            VB = singles.tile([P, DT], F32, tag="VB")
            identity = singles.tile([P, P], F32, tag="ident")
            # [jc, ib] layout; the extra IC row holds -n_pad so the plain
            # row-sum over jc comes out already pad-corrected
            Zbig = singles.tile([P, IC + 1, JB], F32, tag="Zbig")
            diag_cols = singles.tile([P, JB], F32, tag="diag_cols")

            # ---- DMA issue: A first (needed by every Y matmul), then XT
            # chunks round-robin over queues, small singles, then xg rows
            # (only needed at finalize). ----
            qs = (nc.sync, nc.scalar)
            nc.sync.dma_start(AW[:, :, :], aw[:, :])
            qi = 1
            for c0, w in chunks:
                for ei in range(DT):
                    qs[qi % len(qs)].dma_start(
                        XT[:, ei, c0 : c0 + w], xt[ei, :, c0 : c0 + w]
                    )
                    qi += 1
            nc.scalar.dma_start(VB, vb[:, :])
            nc.gpsimd.dma_start(Zbig[:, IC, :], npz[:, :])
            nc.gpsimd.dma_start(identity, idf[:, :])
            # xg rows are needed only at finalize: keep them off the
            # critical queues and gate them behind mid-wavefront YT
            # progress so they don't steal HBM bandwidth from XT at the
            # head
            xg_gate = singles.tile([P, 1], BF16, tag="xg_gate")

            def emit_xg_dmas(gate_c0):
                nc.gpsimd.tensor_copy(out=xg_gate, in_=YT[:, 0, gate_c0 : gate_c0 + 1])
                for ib in range(JB):
                    nc.gpsimd.dma_start(XG[:, ib, :], xg[ib, :, :])

            po1 = psO.tile([1, HD], F32, tag="po1")
            po2 = psO.tile([1, HD], F32, tag="po2")

            def emit_y(c0, w):
                for eo in range(DT):
                    ps = psS.tile([P, CH], F32, tag="psS")
                    for ei in range(DT):
                        nc.tensor.matmul(
                            ps[:, :w],
                            lhsT=AW[:, ei, eo * P : (eo + 1) * P],
                            rhs=XT[:, ei, c0 : c0 + w],
                            start=(ei == 0),
                            stop=(ei == DT - 1),
                        )
                    nc.scalar.activation(
                        out=YT[:, eo, c0 : c0 + w],
                        in_=ps[:, :w],
                        func=mybir.ActivationFunctionType.Identity,
                        bias=VB[:, eo : eo + 1],
                        scale=1.0,
                    )

            def emit_s(ib, jc):
                c0, w = chunks[jc]
                ps = psS.tile([P, CH], F32, tag="psS")
                for et in range(DT):
                    nc.tensor.matmul(
                        ps[:, :w],
                        lhsT=YT[:, et, ib * P : (ib + 1) * P],
                        rhs=XT[:, et, c0 : c0 + w],
                        start=(et == 0),
                        stop=(et == DT - 1),
                    )
                e_sb = spool.tile([P, CH], F32, tag="esb")
                nc.scalar.activation(
                    out=e_sb[:, :w],
                    in_=ps[:, :w],
                    func=mybir.ActivationFunctionType.Exp,
                )
                nc.vector.reduce_sum(
                    Zbig[:, jc, ib : ib + 1], e_sb[:, :w], axis=mybir.AxisListType.X
                )
                dj = ib * P
                if c0 <= dj < c0 + w:
                    off = dj - c0
                    dsc = spool.tile([P, P], F32, tag="dsc")
                    nc.vector.tensor_mul(dsc, e_sb[:, off : off + P], identity)
                    nc.vector.reduce_sum(
                        diag_cols[:, ib : ib + 1], dsc, axis=mybir.AxisListType.X
                    )

            fin_n = [0]

            def emit_finalize(ib):
                z = stats.tile([P, 1], F32, tag="z")
                nc.vector.reduce_sum(
                    z, Zbig[:, :, ib : ib + 1], axis=mybir.AxisListType.XY
                )
                rz = stats.tile([P, 1], F32, tag="rz")
                nc.vector.reciprocal(rz, z)
                wcol = stats.tile([P, 1], BF16, tag="wcol")
                nc.vector.tensor_mul(wcol, diag_cols[:, ib : ib + 1], rz)
                for po, sl in ((po1, slice(0, HD)), (po2, slice(HD, D))):
                    nc.tensor.matmul(
                        po,
                        lhsT=wcol,
                        rhs=XG[:, ib, sl],
                        start=(fin_n[0] == 0),
                        stop=(fin_n[0] == JB - 1),
                    )
                fin_n[0] += 1

            # ---- wavefront: per chunk s compute YT(s), then all S(ib, jc)
            # with max(block(ib), jc) == s ----
            last = len(chunks) - 1
            for s, (c0, w) in enumerate(chunks):
                emit_y(c0, w)
                if s == min(3, last):
                    emit_xg_dmas(c0)
                sb0 = c0 // P
                sb1 = (c0 + w + P - 1) // P
                for ib in range(sb0, sb1):
                    for jc in range(s + 1):
                        emit_s(ib, jc)
                    if s == last:
                        emit_finalize(ib)
                for ib in range(0, sb0):
                    emit_s(ib, s)
                    if s == last:
                        emit_finalize(ib)

            out_sb = singles.tile([1, D], F32, tag="out_sb")
            nc.vector.tensor_copy(out=out_sb[:, :HD], in_=po1)
            nc.vector.tensor_copy(out=out_sb[:, HD:], in_=po2)
            nc.sync.dma_start(out[:, :], out_sb)

    return nc


def prepare_host_inputs(inputs, mask):
    """Per-batch gather + zero-pad to the common padded length Tg."""
    idxs, counts = [], []
    for b in range(B):
        nz = np.nonzero(mask[b])[0]
        idxs.append(nz)
        counts.append(len(nz))
    Tg = max(max(counts), 1)
    Tg = ((Tg + P - 1) // P) * P
    return Tg, idxs, counts


def kernel(inputs, mask, Wq_w, Wq_b, Wk_w, Wk_b, qk_bf16=True, _trace=False):
    from concourse.bass_utils import run_bass_kernel_spmd

    inputs = np.ascontiguousarray(inputs, np.float32)
    mask = np.asarray(mask)
    Tg, idxs, counts = prepare_host_inputs(inputs, mask)
    JB = Tg // P

    nc = bacc.Bacc()
    build_graph(nc, Tg)
    nc.compile()

    # s * Wq^T Wk  and  s * Wk^T bq  (row-constant score terms cancel)
    A = (np.asarray(Wq_w, np.float32).T @ np.asarray(Wk_w, np.float32)) * SCALE
    vvec = (np.asarray(Wk_w, np.float32).T @ np.asarray(Wq_b, np.float32)) * SCALE
    aw_arr = np.ascontiguousarray(
        A.astype(BF).reshape(DT, P, D).transpose(1, 0, 2).reshape(P, DT * D)
    )
    vb_arr = np.ascontiguousarray(vvec.reshape(DT, P).T)
    idf = np.eye(P, dtype=np.float32)

    in_maps = []
    for b in range(B):
        n = counts[b]
        xg_f = np.zeros((Tg, D), np.float32)
        if n:
            xg_f[:n] = inputs[b][idxs[b]]
        xg_bf = xg_f.astype(BF)
        xt_arr = np.ascontiguousarray(xg_bf.T.reshape(DT, P, Tg))
        xg_arr = xg_bf.reshape(JB, P, D)
        in_maps.append(
            {
                "xt": xt_arr,
                "aw": aw_arr,
                "vb": vb_arr,
                "idf": idf,
                "npz": np.full((P, JB), -float(Tg - n), np.float32),
                "xg": xg_arr,
            }
        )

    res = run_bass_kernel_spmd(
        nc, in_maps, core_ids=list(range(NCORES)), trace=_trace
    )
    out = np.stack([res.results[b]["out"][0] for b in range(B)], axis=0)

    # degenerate all-masked batch: softmax over a constant row is uniform
    for b in range(B):
        if counts[b] == 0:
            out[b] = inputs[b].mean(axis=0)

    if _trace:
        return out, res
    return out
